# revision 6
# baseline (speedup 1.0000x reference)
"""Trainium2 Bass kernel for nn_GAT_58935541235964 (2-layer GAT + highway gates).

Strategy (8 NeuronCores, SPMD), destination-node sharding:
  - core c owns nodes [c*12544, (c+1)*12544) of the zero-padded node set
    (100000 -> 100352 = 8 * 98 * 128).
  - Per layer: each core computes its slice of Wh_ext = x @ [W | W@a1 | W@a2]
    (rows padded to 256 f32 = 1KB so dma_gather's 256B granularity holds),
    AllGather replicates Wh_ext, then each core aggregates its own destination
    tiles:
      * per-edge source rows fetched with dma_gather (int16 idx =>
        address-bucketed; slots padded per (tile,bucket) to 128-multiples with
        row-0 fillers whose one-hot column is zero),
      * per-edge destination scores fetched with a 256B sub-row dma_gather
        from the core-local slice,
      * softmax numerator+denominator fused into one matmul per 128-edge chunk:
        psum[128 nodes, 201] += (onehot*exp(lrelu(s)))^T @ [Wh_src | 1],
      * epilogue: gat = sigmoid(num/denom), highway gate GEMM, fused next-layer
        GEMM.

I/O minimization (the axon tunnel runs at ~50-80 MB/s, so bytes dominate):
  - x ships as fp16 rows only; the f32 copy (SWDGE cast-DMA) and the
    transposed layout (tensor-engine transpose) are produced on device.
  - gather indices ship as [16, n] int16 (the dma_gather layout is the same
    16-row block replicated on each 16-partition group; replication happens
    on device with 8 DMA loads).
  - only the batch rows (h/t) leave the device: the final x is written to an
    internal f32 [npc, 256] tensor and dma_gather'ed by per-call batch
    indices into a [128, NBCH*200] fp16 output (~0.5 MB/core vs 10 MB full).
  - the compiled executable and all device-resident inputs are cached across
    kernel() calls; inputs are revalidated with np.array_equal (bitwise) and
    only re-uploaded when they actually change.
"""

import os
import sys
import hashlib

import numpy as np

for _p in ("/opt/trn_rl_repo", "/root/.axon_site/_ro/trn_rl_repo"):
    if os.path.isdir(_p) and _p not in sys.path:
        sys.path.insert(0, _p)

# ---------------------------------------------------------------- config

NCORES = 8
D = 200            # feature dim
ROWW = 256         # padded Wh row width in f32 elems (1KB rows)
ALPHA = 0.01       # leaky relu slope
GG = 7             # tiles per gather group
NBUCK = 5          # int16 address buckets over the padded node set
DENOM_EPS = 1e-9
NPC = 12544        # nodes per core (padded)
NB = 1280          # batch-gather slots per core (h+t combined; mean 1024)
NBCH = NB // 128   # gathered chunks per core

DB = D - 128       # 72

# device-kernel variant flags (bisectable)
FLAGS = dict(fp16_x=True, dev_rep_idx=True, fp16_out=True)


# ---------------------------------------------------------------- host preprocessing

def _preprocess(edge_src, edge_dst, npc=NPC, nbuck=NBUCK, gg=GG):
    """Uniform cross-core slot schedule + per-core index arrays.

    Slot layout (identical on every core): groups of `gg` tiles; within a
    group, chunks are bucket-major: for each bucket b, each tile t contributes
    ceil(max_core_count[t,b]/128) 128-slot chunks.  Real edges fill a
    (tile,bucket) segment first; remaining slots gather row 0 of the bucket
    with dloc=-1 (zero one-hot column => no contribution).

    Index arrays are emitted in the compact [16, n] layout (the dma_gather
    idx layout is the same 16 rows replicated on all 8 16-partition groups).
    """
    tpc = npc // 128
    n_pad = npc * NCORES
    bsz = -(-n_pad // nbuck)               # bucket rows
    assert bsz <= 32768
    edge_src = np.asarray(edge_src, dtype=np.int64)
    edge_dst = np.asarray(edge_dst, dtype=np.int64)

    gtile = edge_dst // 128
    buck = edge_src // bsz
    key = gtile * nbuck + buck
    order = np.argsort(key, kind="stable")
    src_s = edge_src[order]
    dst_s = edge_dst[order]
    ntile = NCORES * tpc
    counts = np.bincount(key[order], minlength=ntile * nbuck)
    starts = np.zeros(ntile * nbuck + 1, dtype=np.int64)
    np.cumsum(counts, out=starts[1:])
    cnt = counts.reshape(NCORES, tpc, nbuck)

    # uniform chunks per (local tile, bucket): max over cores
    ceil_tb = (cnt.max(axis=0) + 127) // 128          # [tpc, nbuck]
    empty = ceil_tb.sum(axis=1) == 0
    ceil_tb[empty, 0] = 1                             # keep >=1 chunk per tile

    groups = []
    ch_tot = 0
    sw_tot = 0
    for g0 in range(0, tpc, gg):
        g1 = min(g0 + gg, tpc)
        kb = ceil_tb[g0:g1].sum(axis=0)               # chunks per bucket [nbuck]
        Kg = int(kb.sum())
        # chunk index within group for (t, b, j)
        choff = {}
        ch = 0
        for b in range(nbuck):
            for t in range(g0, g1):
                if ceil_tb[t, b]:
                    choff[(t, b)] = ch
                    ch += int(ceil_tb[t, b])
        groups.append(dict(t0=g0, t1=g1, Kg=Kg, kb=kb.tolist(), choff=choff,
                           ch_base=ch_tot, sw_base=sw_tot))
        ch_tot += Kg
        sw_tot += 8 * Kg                              # int16 cols for src idx
    schedule = dict(tpc=tpc, npc=npc, nbuck=nbuck, bsz=bsz, ceil_tb=ceil_tb,
                    groups=groups, ch_tot=ch_tot, sw_tot=sw_tot, gg=gg)

    per_core = []
    for c in range(NCORES):
        srcidx = np.zeros((16, sw_tot), dtype=np.int16)
        dstidx = np.zeros((16, 8 * ch_tot), dtype=np.int16)
        dloc = np.full((128, ch_tot), -1.0, dtype=np.float32)
        for g in groups:
            for b in range(nbuck):
                for t in range(g["t0"], g["t1"]):
                    K = int(ceil_tb[t, b])
                    if K == 0:
                        continue
                    ch = g["choff"][(t, b)]           # chunk within group
                    gch = g["ch_base"] + ch           # global chunk
                    gt = (c * tpc + t) * nbuck + b
                    s0, s1 = starts[gt], starts[gt + 1]
                    n = int(s1 - s0)
                    nsl = 128 * K
                    assert n <= nsl
                    bs = np.zeros(nsl, dtype=np.int16)
                    bd = np.zeros(nsl, dtype=np.int16)
                    bl = np.full(nsl, -1.0, dtype=np.float32)
                    bs[:n] = (src_s[s0:s1] - b * bsz).astype(np.int16)
                    bd[:n] = (dst_s[s0:s1] - c * npc).astype(np.int16)
                    bl[:n] = (dst_s[s0:s1] % 128).astype(np.float32)
                    # src idx: 16-wrap at this gather's slot offset
                    soff = g["sw_base"] + 8 * ch      # bucket area within group
                    srcidx[:, soff:soff + nsl // 16] = bs.reshape(nsl // 16, 16).T
                    # dst idx: 16-wrap at the global slot position
                    dstidx[:, 8 * gch:8 * gch + nsl // 16] = bd.reshape(nsl // 16, 16).T
                    dloc[:, gch:gch + K] = bl.reshape(K, 128).T
        per_core.append(dict(srcidx=srcidx, dstidx=dstidx, dloc=dloc))
    return schedule, per_core


# ---------------------------------------------------------------- bass kernel builder

def _build(schedule, fp16_x=True, dev_rep_idx=True, fp16_out=True):
    import concourse.bacc as bacc
    import concourse.mybir as mybir
    import concourse.tile as tile

    F32 = mybir.dt.float32
    F16 = mybir.dt.float16
    I16 = mybir.dt.int16
    A = mybir.AluOpType
    ACT = mybir.ActivationFunctionType

    tpc = schedule["tpc"]
    npc = schedule["npc"]
    nbuck = schedule["nbuck"]
    bsz = schedule["bsz"]
    ceil_tb = schedule["ceil_tb"]
    groups = schedule["groups"]
    ch_tot = schedule["ch_tot"]
    sw_tot = schedule["sw_tot"]
    n_pad = npc * NCORES

    s_src_col = 200                 # f32 col holding s_src in the value row
    d_off, d_elem, sde = 192, 64, 201 - 192

    nc = bacc.Bacc("TRN2", target_bir_lowering=False, debug=False,
                   enable_asserts=True, num_devices=NCORES)

    # ---- I/O
    if fp16_x:
        x_in = nc.dram_tensor("x16", [npc, D], F16, kind="ExternalInput")
    else:
        x_in = nc.dram_tensor("xf", [npc, D], F32, kind="ExternalInput")
    wext_a = [nc.dram_tensor(f"wext{l}_a", [128, ROWW], F32, kind="ExternalInput")
              for l in (1, 2)]
    wext_b = [nc.dram_tensor(f"wext{l}_b", [D - 128, ROWW], F32, kind="ExternalInput")
              for l in (1, 2)]
    whw_a = nc.dram_tensor("whw_a", [128, D], F32, kind="ExternalInput")
    whw_b = nc.dram_tensor("whw_b", [DB + 1, D], F32, kind="ExternalInput")
    iota_in = nc.dram_tensor("iota_in", [128, 128], F32, kind="ExternalInput")
    ident_in = nc.dram_tensor("ident_in", [128, 128], F32, kind="ExternalInput")
    idx_rows = 16 if dev_rep_idx else 128
    srcidx_in = nc.dram_tensor("srcidx", [idx_rows, sw_tot], I16,
                               kind="ExternalInput")
    dstidx_in = nc.dram_tensor("dstidx", [idx_rows, 8 * ch_tot], I16,
                               kind="ExternalInput")
    dloc_in = nc.dram_tensor("dloc", [128, ch_tot], F32, kind="ExternalInput")
    bidx_in = nc.dram_tensor("bidx", [idx_rows, NB // 16], I16,
                             kind="ExternalInput")

    ob_dt = F16 if fp16_out else F32
    ob_out = nc.dram_tensor("ob", [128, NBCH * D], ob_dt, kind="ExternalOutput")

    x0 = nc.dram_tensor("x0", [npc, D], F32, kind="Internal")
    x0T = nc.dram_tensor("x0T", [D, npc], F32, kind="Internal")
    x1 = nc.dram_tensor("x1", [npc, D], F32, kind="Internal")
    x1T = nc.dram_tensor("x1T", [D, npc], F32, kind="Internal")
    xpad = nc.dram_tensor("xpad", [npc, ROWW], F32, kind="Internal")
    cc_in = [nc.dram_tensor(f"cc{l}_in", [npc, ROWW], F32, kind="Internal")
             for l in (1, 2)]
    cc_out = [nc.dram_tensor(f"cc{l}_out", [n_pad, ROWW], F32, kind="Internal",
                             addr_space="Shared") for l in (1, 2)]

    with tile.TileContext(nc) as tc:
        with tc.tile_pool(name="const", bufs=1) as cpool, \
             tc.tile_pool(name="sb", bufs=3) as sb, \
             tc.tile_pool(name="gbuf", bufs=2) as gbuf, \
             tc.tile_pool(name="ps", bufs=2, space="PSUM") as ps:

            # ---- constants
            c_wea = [cpool.tile([128, ROWW], F32, name=f"c_wea{l}") for l in (0, 1)]
            c_web = [cpool.tile([DB, ROWW], F32, name=f"c_web{l}") for l in (0, 1)]
            for l in (0, 1):
                nc.sync.dma_start(c_wea[l][:], wext_a[l][:])
                nc.sync.dma_start(c_web[l][:], wext_b[l][:])
            c_hwa = cpool.tile([128, D], F32)
            c_hwb = cpool.tile([DB + 1, D], F32)
            nc.sync.dma_start(c_hwa[:], whw_a[:])
            nc.sync.dma_start(c_hwb[:], whw_b[:])
            c_iota = cpool.tile([128, 128], F32)
            nc.sync.dma_start(c_iota[:], iota_in[:])
            c_id = cpool.tile([128, 128], F32)
            nc.sync.dma_start(c_id[:], ident_in[:])

            # ---- resident gather indices (replicated on device to 128 rows
            #      when shipped compact)
            c_sidx = cpool.tile([128, sw_tot], I16, name="c_sidx")
            c_didx = cpool.tile([128, 8 * ch_tot], I16, name="c_didx")
            c_bidx = cpool.tile([128, NB // 16], I16, name="c_bidx")
            if dev_rep_idx:
                for k in range(8):
                    nc.sync.dma_start(c_sidx[16 * k:16 * (k + 1), :],
                                      srcidx_in[:])
                    nc.sync.dma_start(c_didx[16 * k:16 * (k + 1), :],
                                      dstidx_in[:])
                    nc.sync.dma_start(c_bidx[16 * k:16 * (k + 1), :],
                                      bidx_in[:])
            else:
                nc.sync.dma_start(c_sidx[:], srcidx_in[:])
                nc.sync.dma_start(c_didx[:], dstidx_in[:])
                nc.sync.dma_start(c_bidx[:], bidx_in[:])
            c_dloc = cpool.tile([128, ch_tot], F32, name="c_dloc")
            nc.sync.dma_start(c_dloc[:], dloc_in[:])

            def gemm_tile(i, lhs_a, lhs_b, layer):
                """Wh tile i = lhsT @ Wext[layer] -> f32 tile, DMA to cc_in."""
                p_wh = ps.tile([128, ROWW], F32, tag="mm", name="p_wh")
                nc.tensor.matmul(p_wh[:], lhs_a[:], c_wea[layer][:],
                                 start=True, stop=False)
                nc.tensor.matmul(p_wh[:], lhs_b[0:DB, :], c_web[layer][:],
                                 start=False, stop=True)
                t_wh = sb.tile([128, ROWW], F32, tag="whsb", name="t_wh")
                nc.scalar.copy(t_wh[:, 0:202], p_wh[:, 0:202])
                nc.vector.memset(t_wh[:, 202:ROWW], 0.0)
                nc.sync.dma_start(cc_in[layer][i * 128:(i + 1) * 128, :],
                                  t_wh[:])

            # ===== phase G1: load/cast x, build x0/x0T on device, layer-1 GEMM
            for i in range(tpc):
                t_xr = sb.tile([128, D], F32, tag="xr", name="t_xr")
                if fp16_x:
                    # SWDGE casts fp16 -> f32 during the DMA
                    nc.gpsimd.dma_start(t_xr[:], x_in[i * 128:(i + 1) * 128, :])
                else:
                    nc.sync.dma_start(t_xr[:], x_in[i * 128:(i + 1) * 128, :])
                nc.sync.dma_start(x0[i * 128:(i + 1) * 128, :], t_xr[:])
                p_t1 = ps.tile([128, 128], F32, tag="tr", name="p_t1")
                nc.tensor.transpose(p_t1[:], t_xr[:, 0:128], c_id[:])
                p_t2 = ps.tile([128, 128], F32, tag="tr", name="p_t2")
                nc.tensor.transpose(p_t2[0:DB, :], t_xr[:, 128:D], c_id[:])
                xt_a = sb.tile([128, 128], F32, tag="xt_a", name="xt_a")
                nc.scalar.copy(xt_a[:], p_t1[:])
                xt_b = sb.tile([DB, 128], F32, tag="xt_b", name="xt_b")
                nc.scalar.copy(xt_b[:], p_t2[0:DB, :])
                nc.sync.dma_start(x0T[0:128, i * 128:(i + 1) * 128], xt_a[:])
                nc.sync.dma_start(x0T[128:D, i * 128:(i + 1) * 128], xt_b[:])
                gemm_tile(i, xt_a, xt_b, 0)

            # ===== per-layer aggregation
            def group_loads(g, layer):
                Kg = g["Kg"]
                kb = g["kb"]
                chb, swb = g["ch_base"], g["sw_base"]

                t_G = gbuf.tile([128, Kg, ROWW], F32, tag="G", name="t_G")
                c0 = 0
                for b in range(nbuck):
                    Kb = int(kb[b])
                    if Kb == 0:
                        continue
                    nrows = min(bsz, n_pad - b * bsz)
                    for cs in range(0, Kb, 8):
                        kk = min(8, Kb - cs)
                        nc.gpsimd.dma_gather(
                            out_ap=t_G[:, c0 + cs:c0 + cs + kk, :],
                            in_ap=cc_out[layer][b * bsz:b * bsz + nrows, :],
                            idxs_ap=c_sidx[:, swb + 8 * (c0 + cs):
                                           swb + 8 * (c0 + cs + kk)],
                            num_idxs=128 * kk, num_idxs_reg=128 * kk,
                            elem_size=ROWW)
                    c0 += Kb
                t_Gd = gbuf.tile([128, Kg, d_elem], F32, tag="Gd",
                                 name="t_Gd")
                for cs in range(0, Kg, 8):
                    kk = min(8, Kg - cs)
                    nc.gpsimd.dma_gather(
                        out_ap=t_Gd[:, cs:cs + kk, :],
                        in_ap=cc_in[layer][:, d_off:ROWW],
                        idxs_ap=c_didx[:, 8 * (chb + cs):8 * (chb + cs + kk)],
                        num_idxs=128 * kk, num_idxs_reg=128 * kk,
                        elem_size=d_elem, elem_step=ROWW)
                return t_G, t_Gd

            def aggregation(layer, x_rows, xT_src, x_next, do_next_gemm,
                            pad_out=None):
                for g in groups:
                    t0, t1, Kg = g["t0"], g["t1"], g["Kg"]
                    chb = g["ch_base"]
                    choff = g["choff"]
                    t_G, t_Gd = group_loads(g, layer)

                    # group-wide edge scores: ex = exp(lrelu(s_src + s_dst))
                    t_sc = sb.tile([128, Kg, 1], F32, tag="sc", name="t_sc")
                    ssrc = t_G[:, 0:Kg, s_src_col:s_src_col + 1]
                    sdst = t_Gd[:, 0:Kg, sde:sde + 1]
                    nc.vector.tensor_tensor(t_sc[:], ssrc, sdst, A.add)
                    t_lr = sb.tile([128, Kg, 1], F32, tag="lr", name="t_lr")
                    nc.vector.scalar_tensor_tensor(
                        out=t_lr[:], in0=t_sc[:], scalar=ALPHA,
                        in1=t_sc[:], op0=A.mult, op1=A.max)
                    t_ex = sb.tile([128, Kg, 1], F32, tag="ex", name="t_ex")
                    nc.scalar.activation(t_ex[:], t_lr[:], ACT.Exp)

                    for t in range(t0, t1):
                        chunks = [(choff[(t, b)] + j, b)
                                  for b in range(nbuck) if ceil_tb[t, b]
                                  for j in range(int(ceil_tb[t, b]))]
                        p_agg = ps.tile([128, 201], F32, tag="agg", name="p_agg")
                        for kk, (ch, _b) in enumerate(chunks):
                            t_oh = sb.tile([128, 128], F32, tag="oh", name="t_oh")
                            nc.vector.tensor_scalar(
                                out=t_oh[:], in0=c_iota[:],
                                scalar1=c_dloc[:, chb + ch:chb + ch + 1],
                                scalar2=t_ex[:, ch, :],
                                op0=A.is_equal, op1=A.mult)
                            nc.vector.memset(
                                t_G[:, ch, s_src_col:s_src_col + 1], 1.0)
                            nc.tensor.matmul(
                                p_agg[:], t_oh[:],
                                t_G[:, ch, 0:s_src_col + 1],
                                start=(kk == 0), stop=(kk == len(chunks) - 1))

                        # epilogue: gat = sigmoid(num * recip(max(den, eps)))
                        t_den = sb.tile([128, 1], F32, tag="den", name="t_den")
                        nc.vector.tensor_scalar_max(t_den[:], p_agg[:, 200:201],
                                                    DENOM_EPS)
                        t_rd = sb.tile([128, 1], F32, tag="rd", name="t_rd")
                        nc.vector.reciprocal(t_rd[:], t_den[:])
                        t_gat = sb.tile([128, D], F32, tag="gat", name="t_gat")
                        nc.scalar.activation(t_gat[:], p_agg[:, 0:D],
                                             ACT.Sigmoid, bias=0.0,
                                             scale=t_rd[:])

                        # highway: sigma = sigmoid(x @ W_hw + b)
                        t_x = sb.tile([128, D], F32, tag="x", name="t_x")
                        nc.sync.dma_start(t_x[:],
                                          x_rows[t * 128:(t + 1) * 128, :])
                        t_xta = sb.tile([128, 128], F32, tag="xta", name="t_xta")
                        nc.sync.dma_start(t_xta[:],
                                          xT_src[0:128, t * 128:(t + 1) * 128])
                        t_xtb = sb.tile([DB + 1, 128], F32, tag="xtb",
                                        name="t_xtb")
                        nc.vector.memset(t_xtb[:], 1.0)
                        nc.sync.dma_start(t_xtb[0:DB, :],
                                          xT_src[128:D, t * 128:(t + 1) * 128])
                        p_sig = ps.tile([128, D], F32, tag="mm", name="p_sig")
                        nc.tensor.matmul(p_sig[:], t_xta[:], c_hwa[:],
                                         start=True, stop=False)
                        nc.tensor.matmul(p_sig[:], t_xtb[:], c_hwb[:],
                                         start=False, stop=True)
                        t_sig = sb.tile([128, D], F32, tag="sig", name="t_sig")
                        nc.scalar.activation(t_sig[:], p_sig[:], ACT.Sigmoid)

                        # x_new = x + sigma * (gat - x)
                        t_dif = sb.tile([128, D], F32, tag="dif", name="t_dif")
                        nc.vector.tensor_sub(t_dif[:], t_gat[:], t_x[:])
                        t_sd = sb.tile([128, D], F32, tag="sd", name="t_sd")
                        nc.vector.tensor_mul(t_sd[:], t_sig[:], t_dif[:])
                        t_xn = sb.tile([128, D], F32, tag="xn", name="t_xn")
                        nc.vector.tensor_add(t_xn[:], t_x[:], t_sd[:])
                        if pad_out is not None:
                            nc.sync.dma_start(
                                pad_out[t * 128:(t + 1) * 128, 0:D], t_xn[:])
                        else:
                            nc.sync.dma_start(x_next[t * 128:(t + 1) * 128, :],
                                              t_xn[:])

                        if do_next_gemm:
                            p_n1 = ps.tile([128, 128], F32, tag="tr", name="p_n1")
                            nc.tensor.transpose(p_n1[:], t_xn[:, 0:128], c_id[:])
                            p_n2 = ps.tile([128, 128], F32, tag="tr", name="p_n2")
                            nc.tensor.transpose(p_n2[0:DB, :], t_xn[:, 128:D],
                                                c_id[:])
                            t_na = sb.tile([128, 128], F32, tag="xt_a",
                                           name="t_na")
                            nc.scalar.copy(t_na[:], p_n1[:])
                            t_nb = sb.tile([DB, 128], F32, tag="xt_b",
                                           name="t_nb")
                            nc.scalar.copy(t_nb[:], p_n2[0:DB, :])
                            nc.sync.dma_start(
                                x1T[0:128, t * 128:(t + 1) * 128], t_na[:])
                            nc.sync.dma_start(
                                x1T[128:D, t * 128:(t + 1) * 128], t_nb[:])
                            gemm_tile(t, t_na, t_nb, 1)

            import concourse.mybir as _mb
            # layer 1
            nc.gpsimd.collective_compute(
                "AllGather", _mb.AluOpType.bypass,
                replica_groups=[list(range(NCORES))],
                ins=[cc_in[0][:]], outs=[cc_out[0][:]])
            aggregation(0, x0, x0T, x1, do_next_gemm=True)
            # layer 2
            nc.gpsimd.collective_compute(
                "AllGather", _mb.AluOpType.bypass,
                replica_groups=[list(range(NCORES))],
                ins=[cc_in[1][:]], outs=[cc_out[1][:]])
            aggregation(1, x1, x1T, None, do_next_gemm=False, pad_out=xpad)

            # ===== final: gather the batch rows from xpad (f32, 1KB rows)
            t_gb = sb.tile([128, NBCH, ROWW], F32, tag="gb", name="t_gb")
            for cs in range(0, NBCH, 8):
                kk = min(8, NBCH - cs)
                nc.gpsimd.dma_gather(
                    out_ap=t_gb[:, cs:cs + kk, :], in_ap=xpad[:],
                    idxs_ap=c_bidx[:, 8 * cs:8 * (cs + kk)],
                    num_idxs=128 * kk, num_idxs_reg=128 * kk,
                    elem_size=ROWW)
            for j in range(NBCH):
                if fp16_out:
                    # SWDGE casts f32 -> fp16 during the DMA
                    nc.gpsimd.dma_start(ob_out[:, j * D:(j + 1) * D],
                                        t_gb[:, j, 0:D])
                else:
                    nc.sync.dma_start(ob_out[:, j * D:(j + 1) * D],
                                      t_gb[:, j, 0:D])

    nc.finalize()
    return nc


# ---------------------------------------------------------------- weights

def _prepare_weights(W_gat, att_a, W_hw, b_hw):
    """Fold attention projections into padded GEMM weights (host-side layout)."""
    outs = []
    for l in range(2):
        wext = np.zeros((D, ROWW), np.float32)
        wext[:, :D] = W_gat[l]
        a1 = att_a[l][:D].astype(np.float64)
        a2 = att_a[l][D:].astype(np.float64)
        wext[:, 200] = (W_gat[l].astype(np.float64) @ a1).astype(np.float32)
        wext[:, 201] = (W_gat[l].astype(np.float64) @ a2).astype(np.float32)
        outs.append(wext)
    whw_a = W_hw[0:128].astype(np.float32)
    whw_b = np.concatenate([W_hw[128:D], b_hw.reshape(1, D)], 0).astype(np.float32)
    return outs, whw_a, whw_b


# ---------------------------------------------------------------- runner

class _Runner:
    """Holds the compiled executable + device-resident inputs for one edge set."""

    def __init__(self, edge_src, edge_dst, flags=None):
        import jax
        from jax.sharding import Mesh, PartitionSpec, NamedSharding
        from jax.experimental.shard_map import shard_map
        from concourse import mybir
        from concourse.bass2jax import (_bass_exec_p, install_neuronx_cc_hook,
                                        partition_id_tensor)

        self.flags = dict(FLAGS if flags is None else flags)
        self.jax = jax
        self.schedule, self.per_core = _preprocess(edge_src, edge_dst)
        self.nc = _build(self.schedule, **self.flags)
        install_neuronx_cc_hook()

        nc = self.nc
        partition_name = (nc.partition_id_tensor.name
                          if nc.partition_id_tensor else None)
        in_names, out_names, out_avals = [], [], []
        for alloc in nc.m.functions[0].allocations:
            if not isinstance(alloc, mybir.MemoryLocationSet):
                continue
            name = alloc.memorylocations[0].name
            if alloc.kind == "ExternalInput":
                if name != partition_name:
                    in_names.append(name)
            elif alloc.kind == "ExternalOutput":
                out_names.append(name)
                out_avals.append(jax.core.ShapedArray(
                    tuple(alloc.tensor_shape), mybir.dt.np(alloc.dtype)))
        self.in_names = in_names
        self.out_names = out_names
        self.out_avals = out_avals
        n_params = len(in_names)
        n_outs = len(out_names)
        in_names_all = in_names + out_names + (
            [partition_name] if partition_name else [])

        def _body(*args):
            operands = list(args)
            if partition_name is not None:
                operands.append(partition_id_tensor())
            return tuple(_bass_exec_p.bind(
                *operands, out_avals=tuple(out_avals),
                in_names=tuple(in_names_all), out_names=tuple(out_names),
                lowering_input_output_aliases=(),
                sim_require_finite=True, sim_require_nnan=True, nc=nc))

        devices = jax.devices()[:NCORES]
        assert len(devices) == NCORES
        mesh = Mesh(np.asarray(devices), ("core",))
        self.sh = NamedSharding(mesh, PartitionSpec("core"))
        self.sharded = jax.jit(
            shard_map(_body, mesh=mesh,
                      in_specs=(PartitionSpec("core"),) * (n_params + n_outs),
                      out_specs=(PartitionSpec("core"),) * n_outs,
                      check_rep=False),
            keep_unused=True)

        # device-resident static inputs (per edge set)
        iota = np.tile(np.arange(128, dtype=np.float32)[None, :], (128, 1))
        ident = np.eye(128, dtype=np.float32)
        self.dev = {}
        self._put_same("iota_in", iota)
        self._put_same("ident_in", ident)
        rep = 1 if self.flags["dev_rep_idx"] else 8
        self._put_concat("srcidx", [np.tile(pc["srcidx"], (rep, 1))
                                    for pc in self.per_core])
        self._put_concat("dstidx", [np.tile(pc["dstidx"], (rep, 1))
                                    for pc in self.per_core])
        self._put_concat("dloc", [pc["dloc"] for pc in self.per_core])
        # output placeholder operands (never donated, so upload once)
        self.dev_zeros = [
            jax.device_put(np.zeros((NCORES * a.shape[0], *a.shape[1:]),
                                    a.dtype), self.sh)
            for a in out_avals]
        self._w_key = None
        self._x_key = None
        self._b_key = None

    def _put_same(self, name, arr):
        """Replicate one per-core array across the 8 shards."""
        big = np.broadcast_to(arr, (NCORES, *arr.shape)).reshape(
            NCORES * arr.shape[0], *arr.shape[1:])
        self.dev[name] = self.jax.device_put(np.ascontiguousarray(big), self.sh)

    def _put_concat(self, name, arrs):
        big = np.concatenate(arrs, axis=0)
        self.dev[name] = self.jax.device_put(np.ascontiguousarray(big), self.sh)

    def set_weights(self, W_gat, att_a, W_hw, b_hw):
        key = (W_gat, att_a, W_hw, b_hw)
        if self._w_key is not None and all(
                a is b or np.array_equal(a, b)
                for a, b in zip(self._w_key, key)):
            return
        (wext1, wext2), whw_a, whw_b = _prepare_weights(W_gat, att_a,
                                                        W_hw, b_hw)
        self._put_same("wext1_a", wext1[0:128])
        self._put_same("wext1_b", wext1[128:D])
        self._put_same("wext2_a", wext2[0:128])
        self._put_same("wext2_b", wext2[128:D])
        self._put_same("whw_a", whw_a)
        self._put_same("whw_b", whw_b)
        self._w_key = tuple(np.array(a, copy=True) for a in key)

    def set_x(self, ent_embed):
        if self._x_key is not None and np.array_equal(self._x_key, ent_embed):
            return
        n_pad = NPC * NCORES
        dt = np.float16 if self.flags["fp16_x"] else np.float32
        xs = np.zeros((n_pad, D), dt)
        xs[:ent_embed.shape[0]] = ent_embed.astype(dt)
        self.dev["x16" if self.flags["fp16_x"] else "xf"] = \
            self.jax.device_put(xs, self.sh)
        self._x_key = np.array(ent_embed, copy=True)

    def run_batch(self, comb_idx):
        """Gather rows comb_idx (flat node ids) of the final x; returns
        [len(comb_idx), D] float32."""
        n = len(comb_idx)
        core = comb_idx // NPC
        counts = np.bincount(core, minlength=NCORES)
        if counts.max() > NB:
            # pathological batch skew: recurse on halves (device x is memoized,
            # so each half is just one more exec)
            h = n // 2
            return np.concatenate(
                [self.run_batch(comb_idx[:h]), self.run_batch(comb_idx[h:])], 0)
        if self._b_key is None or not np.array_equal(self._b_key, comb_idx):
            loc = (comb_idx % NPC).astype(np.int16)
            order = np.argsort(core, kind="stable")
            bidx = np.zeros((NCORES, NB), np.int16)
            slot_of = np.empty(n, np.int64)    # global slot per position
            off = 0
            for c in range(NCORES):
                k = counts[c]
                pos = order[off:off + k]
                bidx[c, :k] = loc[pos]
                slot_of[pos] = c * NB + np.arange(k)
                off += k
            bidx16 = np.ascontiguousarray(
                bidx.reshape(NCORES, NB // 16, 16).transpose(0, 2, 1))
            if not self.flags["dev_rep_idx"]:
                bidx16 = np.tile(bidx16, (1, 8, 1))
            self.dev["bidx"] = self.jax.device_put(
                bidx16.reshape(-1, NB // 16), self.sh)
            self._slot_of = slot_of
            self._b_key = np.array(comb_idx, copy=True)

        args = [self.dev[name] for name in self.in_names] + self.dev_zeros
        out = self.sharded(*args)
        ob = np.asarray(out[self.out_names.index("ob")])  # [8*128, NBCH*D]
        rows = ob.reshape(NCORES, 128, NBCH, D).transpose(0, 2, 1, 3).reshape(
            NCORES * NB, D)                     # [core*NB + slot, D]
        return rows[self._slot_of].astype(np.float32)


_RUNNERS = {}


def _get_runner(edge_src, edge_dst, flags=None):
    key = (hashlib.sha256(
        np.ascontiguousarray(edge_src).tobytes()
        + np.ascontiguousarray(edge_dst).tobytes()).hexdigest(),
        tuple(sorted((FLAGS if flags is None else flags).items())))
    if key not in _RUNNERS:
        _RUNNERS[key] = _Runner(edge_src, edge_dst, flags)
    return _RUNNERS[key]


def kernel(ent_embed, rel_embed, W_gat, att_a, W_hw, b_hw,
           edge_src, edge_dst, batch_h, batch_r, batch_t):
    ent_embed = np.asarray(ent_embed, dtype=np.float32)
    rel_embed = np.asarray(rel_embed, dtype=np.float32)
    W_gat = np.asarray(W_gat, dtype=np.float32)
    att_a = np.asarray(att_a, dtype=np.float32)
    W_hw = np.asarray(W_hw, dtype=np.float32)
    b_hw = np.asarray(b_hw, dtype=np.float32)
    edge_src = np.asarray(edge_src, dtype=np.int64)
    edge_dst = np.asarray(edge_dst, dtype=np.int64)
    bh = np.asarray(batch_h, dtype=np.int64)
    br = np.asarray(batch_r, dtype=np.int64)
    bt = np.asarray(batch_t, dtype=np.int64)

    runner = _get_runner(edge_src, edge_dst)
    runner.set_weights(W_gat, att_a, W_hw, b_hw)
    runner.set_x(ent_embed)

    comb = np.concatenate([bh, bt])
    rows = runner.run_batch(comb)
    h = rows[:len(bh)]
    t = rows[len(bh):]
    r = rel_embed[br]
    return (h, r, t)
